# revision 28
# baseline (speedup 1.0000x reference)
"""DynamicGCN edge-MLP message passing kernel for 8x trn2 NeuronCores.

Shapes (hardcoded): x [2, 512, 256] f32, base_adj [2, 512, 512] f32,
W1 [512, 256], b1 [256], W2 [256, 128], b2 [128], W3 [128, 1], b3 [1],
Wg [256, 256], bg [256].  Output [2, 512, 256] f32.

Sharding: core c handles batch b = c // 4 and query rows
i in [128*(c%4), 128*(c%4)+128).  Params replicated; all per-core
variation is input data (same NEFF on all 8 cores).

Per core (i-block of 128 query rows, j = all 512 nodes):
  leftb[c,i]  = (x_i @ W1[:D])[c] + b1[c]          (PE prologue)
  rightT[c,j] = (x_j @ W1[D:])[c]                  (PE prologue, bf16)
  per i (2-row pipelined groups, LAG=4):
    hT[c,j]   = relu(rightT[c,j] + leftb[c,i])     (DVE dual-scalar op,
                  bf16 4x mode; a slice of ops offloaded to Pool)
    h2p[k,j]  = sum_c hT[c,j] * (S*W2*|W3|)[c,k]   (PE, 2 accum matmuls)
    t[k,j]    = relu(h2p[k,j] + S*|W3_k| b2_k)     (psum->sbuf fp8e4;
                  mostly ACT, 1/8 DVE; S=32 keeps t in fp8 normal range)
    edge[i,j] += sum_k (sign(W3_k)/S) t[k,j]       (ONE fp8 DoubleRow
                  matmul per 2-row group: k-tile u carries row u's t and
                  the shifted sign columns route them to psum columns
                  r0/r0+1.  DoubleRow outputs must sit at PE column
                  position 0, <=64 wide, so both 64-row blocks write
                  partitions 0-63 of separate psum banks; the row-half
                  relocation to partitions 64-127 happens for free in the
                  tail transposes.)
  edge[i,:] == h2(i,j,:) @ W3 exactly (|W3| and S folded into W2/b2, sign
  into the reduction weights; relu is positively homogeneous)
  s = tanh(.5*edge + .5*b3)  => sigmoid without a table switch
  adj = .5*badj*(1+s) + eye = .5*badj*s + (.5*badj + eye)
  adjn = softmax(adj) rowwise; normalized in-place to bf16 (half 0 runs
  under the second half of the main loop; bha/bhae for rows 64-127 are
  host-packed onto partitions 0-63)
  out = (adjn @ x_b) @ Wg + bg  (bf16 PE transposes + 2 matmul chains)

Inputs are packed host-side into 4 DRAM tensors (bf16 critical, f32
critical, f32 late, fp8 signs) so the prologue is 4 large DMAs.
"""

import ml_dtypes
import numpy as np

import concourse.bacc as bacc
import concourse.bass as bass
import concourse.mybir as mybir
import concourse.tile as tile
from concourse.bass_utils import run_bass_kernel_spmd

F32 = mybir.dt.float32
BF16 = mybir.dt.bfloat16
FP8 = mybir.dt.float8e4
AF = mybir.ActivationFunctionType
OP = mybir.AluOpType
DR = mybir.MatmulPerfMode.DoubleRow

P = 128      # partitions / i-block
N = 512      # nodes (j dim)
D = 256      # input dim
H = 256      # hidden (c dim, 2 partition tiles)
H2 = 128     # second hidden (k dim)
NCORES = 8
LE = 3       # eviction lag behind h2 production (in 2-row groups)
LD = 6       # DoubleRow-reduction lag; > LE so the PE never waits on a
             # just-in-time ACT eviction
NG = P // 2

# --- packed-input layouts (free-dim element offsets) ---
# bigh (bf16): xt[2]@512, w1b[2]@256, w2s[2]@128, xb8[4]@256, wg8[2]@256,
#   identb@64
BH_XT = 0
BH_W1B = 1024
BH_W2S = 1536
BH_XB8 = 1792
BH_WG8 = 2816
BH_IDB = 3328
BH_TOT = 3392
# bigf (f32, critical): xti[2]@128, w1a[2]@256, b1c@2, b2sc@1, halfb3@1
BF_XTI = 0
BF_W1A = 256
BF_B1C = 768
BF_B2SC = 770
BF_HB3 = 771
BF_TOT = 772
# bigl (f32, late): bhaH@1024, bhaeH@1024, bgt@256
#   bhaH/bhaeH are [64, 2, 512]: half h of the adjacency rows packed onto
#   partitions 0-63 (partitions 64-127 unused) to match the DoubleRow edge
#   output placement.
BL_BHA = 0
BL_BHAE = 1024
BL_BGT = 2048
BL_TOT = 2304
# big8 (fp8e4): sgn2 [P, 2, 128]: tile0 col 64 and tile1 col 65 hold
# sign(w3)/S; window-sliced per group.
B8_TOT = 256
# scale folded into W2/b2 so t = S*relu(...) sits in fp8e4's normal range;
# the sign reduction weights carry 1/S.
TSCALE = 32.0


def _build_program(reps=1):
    """reps>1 wraps the whole kernel body in a For_i loop — used only by
    the timing bench (wall-clock regression over reps)."""
    import contextlib

    nc = bacc.Bacc("TRN2", target_bir_lowering=False, debug=False)

    bigh = nc.dram_tensor("bigh", [P, BH_TOT], BF16, kind="ExternalInput").ap()
    bigf = nc.dram_tensor("bigf", [P, BF_TOT], F32, kind="ExternalInput").ap()
    bigl = nc.dram_tensor("bigl", [P, BL_TOT], F32, kind="ExternalInput").ap()
    big8 = nc.dram_tensor("big8", [P, B8_TOT], FP8, kind="ExternalInput").ap()
    out_d = nc.dram_tensor("out", [P, D], F32, kind="ExternalOutput").ap()

    with tile.TileContext(nc) as tc:
        with (
            tc.tile_pool(name="const", bufs=1) as const,
            tc.tile_pool(name="work", bufs=8) as work,
            tc.tile_pool(name="pedge", bufs=1, space="PSUM") as pedge,
        ):
            bh = const.tile([P, BH_TOT], BF16)
            bf = const.tile([P, BF_TOT], F32)
            bl = const.tile([P, BL_TOT], F32)
            b8 = const.tile([P, 2, 128], FP8)
            loop_cm = tc.For_i(0, reps, 1) if reps > 1 else contextlib.nullcontext()
            loop_cm.__enter__()
            # two critical input DMAs on separate queue engines; the tail-only
            # half of bigh (xb8/wg8/identb) and bigl follow behind
            nc.sync.dma_start(bh[:, :BH_XB8], bigh[:, :BH_XB8])
            nc.gpsimd.dma_start(bf[:], bigf)
            nc.sync.dma_start(b8[:], big8)
            nc.sync.dma_start(bh[:, BH_XB8:], bigh[:, BH_XB8:])
            nc.gpsimd.dma_start(bl[:], bigl)

            def xt_sb(dt):
                return bh[:, BH_XT + 512 * dt : BH_XT + 512 * (dt + 1)]

            def w1b_sb(dt, csl):
                base = BH_W1B + 256 * dt
                return bh[:, base + csl * 128 : base + csl * 128 + 128]

            def w2s_sb(ct):
                return bh[:, BH_W2S + 128 * ct : BH_W2S + 128 * (ct + 1)]

            def xb8_sb(jt, dh):
                base = BH_XB8 + 256 * jt + 128 * dh
                return bh[:, base : base + 128]

            def wg8_sb(dt):
                return bh[:, BH_WG8 + 256 * dt : BH_WG8 + 256 * (dt + 1)]

            identb_sb = bh[0:64, BH_IDB : BH_IDB + 64]

            def xti_sb(dt):
                return bf[:, BF_XTI + 128 * dt : BF_XTI + 128 * (dt + 1)]

            def w1a_sb(dt, csl):
                base = BF_W1A + 256 * dt
                return bf[:, base + csl * 128 : base + csl * 128 + 128]

            b1c_sb = bf[:, BF_B1C : BF_B1C + 2]
            b2sc_sb = bf[:, BF_B2SC : BF_B2SC + 1]
            halfb3_sb = bf[0:64, BF_HB3 : BF_HB3 + 1]

            def bhaH_sb(h):
                base = BL_BHA + 512 * h
                return bl[0:64, base : base + 512]

            def bhaeH_sb(h):
                base = BL_BHAE + 512 * h
                return bl[0:64, base : base + 512]

            bgt_sb = bl[:, BL_BGT : BL_BGT + 256]

            # Preload the exp/tanh/relu activation table set early so the
            # ~2.7us table DMA overlaps the input DMAs.
            warm = const.tile([P, 1], F32)
            nc.vector.memset(warm[:], 0.0)
            nc.scalar.activation(warm[:], warm[:], AF.Exp)

            # edge logits: 64-row halves in separate PSUM banks, BOTH at
            # partitions 0-63 (DoubleRow dst must sit at PE column position
            # 0); half 0's softmax chain runs while the PE accumulates
            # half 1.
            edge_ps_a = pedge.tile([64, N], F32, tag="edgea")
            edge_ps_b = pedge.tile([64, N], F32, tag="edgeb")
            edge_banks = (edge_ps_a, edge_ps_b)
            scratch_sb = const.tile([P, N], BF16)
            nc.vector.memset(scratch_sb[:], 0.0)

            with tc.tile_pool(name="ph2", bufs=3, space="PSUM") as ph2:
                # Warm the PE HAM clock-gate (~3.4us of dummy matmul activity
                # with no DMA dependency) while the input DMAs are in flight,
                # so the real prologue runs at 2.4 GHz instead of 1.2.
                wps = ph2.tile([P, 2, N], F32, tag="h2")
                for w in range(8):
                    nc.tensor.matmul(
                        wps[:, 0, :], scratch_sb[:, :P], scratch_sb[:],
                        start=True, stop=True,
                    )
                nc.vector.tensor_copy(warm[:], wps[:, 0, 0:1])

                # ---- prologue: rightT / leftb ----
                rightT_sb = const.tile([P, 2, N], BF16)
                leftb_sb = const.tile([P, 2, P], F32)
                for ct in range(2):
                    ps = ph2.tile([P, 2, N], F32, tag="h2")
                    for dt in range(2):
                        nc.tensor.matmul(
                            ps[:, 0, :],
                            w1b_sb(dt, ct),
                            xt_sb(dt),
                            start=(dt == 0),
                            stop=(dt == 1),
                        )
                    # one eviction per engine so they run in parallel
                    if ct == 0:
                        nc.scalar.copy(rightT_sb[:, ct, :], ps[:, 0, :])
                    else:
                        nc.vector.tensor_copy(rightT_sb[:, ct, :], ps[:, 0, :])
                for ct in range(2):
                    ps = ph2.tile([P, 2, N], F32, tag="h2")
                    for dt in range(2):
                        nc.tensor.matmul(
                            ps[:, 0, :P],
                            w1a_sb(dt, ct),
                            xti_sb(dt),
                            start=(dt == 0),
                            stop=(dt == 1),
                        )
                    if ct == 0:
                        nc.scalar.activation(
                            leftb_sb[:, ct, :], ps[:, 0, :P], AF.Identity,
                            bias=b1c_sb[:, ct : ct + 1], scale=1.0,
                        )
                    else:
                        nc.vector.tensor_scalar(
                            leftb_sb[:, ct, :], ps[:, 0, :P],
                            b1c_sb[:, ct : ct + 1], None, op0=OP.add,
                        )

                # softmax tiles (both halves live at partitions 0-63)
                s64 = const.tile([64, N], F32)
                m1 = const.tile([64, N], F32)
                m2 = const.tile([64, N], F32)
                adjexp = const.tile([64, 2, N], F32)
                adjn = const.tile([64, 2, N], BF16)
                rowsum = const.tile([64, 2], F32)
                invs = const.tile([64, 2], F32)

                def softmax_half(h, mid):
                    eb = edge_banks[h]
                    nc.scalar.activation(
                        s64[:], eb[:], AF.Tanh,
                        bias=halfb3_sb, scale=0.5,
                    )
                    nc.vector.tensor_tensor(
                        m1[:], s64[:], bhaH_sb(h), op=OP.mult
                    )
                    nc.vector.tensor_tensor(
                        m2[:], m1[:], bhaeH_sb(h), op=OP.add
                    )
                    nc.scalar.activation(
                        adjexp[:, h, :], m2[:], AF.Exp,
                        accum_out=rowsum[:, h : h + 1],
                    )
                    nc.vector.reciprocal(invs[:, h : h + 1], rowsum[:, h : h + 1])
                    nc.vector.tensor_scalar(
                        adjn[:, h, :], adjexp[:, h, :],
                        invs[:, h : h + 1], None, op0=OP.mult,
                    )

                # ---- main loop over the 128 query rows, 2 rows per group ----
                h2ps = {}
                t_sbs = {}
                for step in range(NG + LD):
                    if step < NG:
                        g = step
                        hts = []
                        for u in range(2):
                            i = 2 * g + u
                            ht0 = work.tile([P, N], BF16, tag=f"ht0{u}")
                            ht1 = work.tile([P, N], BF16, tag=f"ht1{u}")
                            nc.vector.tensor_scalar(
                                ht0[:], rightT_sb[:, 0, :],
                                leftb_sb[:, 0, i : i + 1], 0.0,
                                op0=OP.add, op1=OP.max,
                            )
                            nc.vector.tensor_scalar(
                                ht1[:], rightT_sb[:, 1, :],
                                leftb_sb[:, 1, i : i + 1], 0.0,
                                op0=OP.add, op1=OP.max,
                            )
                            hts.append((ht0, ht1))
                        ps = ph2.tile([P, 2, N], F32, tag="h2")
                        for u in range(2):
                            nc.tensor.matmul(
                                ps[:, u, :], w2s_sb(0), hts[u][0][:],
                                start=True, stop=False,
                            )
                        for u in range(2):
                            nc.tensor.matmul(
                                ps[:, u, :], w2s_sb(1), hts[u][1][:],
                                start=False, stop=True,
                            )
                        h2ps[g] = ps
                    ge = step - LE
                    if 0 <= ge < NG:
                        t_sb = work.tile([P, 2, N], FP8, tag="tt")
                        # psum eviction mostly on ACT (GPSIMD has no PSUM
                        # access): real-HW DVE is hT-bound, so it only takes
                        # a few evictions plus alternation in the drain phase
                        if ge % 32 == 15 or (ge >= NG - LE and ge % 2 == 0):
                            nc.vector.tensor_scalar(
                                t_sb[:], h2ps.pop(ge)[:],
                                b2sc_sb[:], 0.0, op0=OP.add, op1=OP.max,
                            )
                        else:
                            nc.scalar.activation(
                                t_sb[:], h2ps.pop(ge)[:], AF.Relu,
                                bias=b2sc_sb[:], scale=1.0,
                            )
                        t_sbs[ge] = t_sb
                    gd = step - LD
                    if 0 <= gd < NG:
                        # one fp8 DoubleRow matmul reduces both rows of the
                        # group: k-tile u carries row u's t, the sign weights
                        # route tile0 -> column r0, tile1 -> r0+1
                        i0 = 2 * gd
                        blk, r0 = divmod(i0, 64)
                        nc.tensor.matmul(
                            edge_banks[blk][:, :],
                            b8[:, :, 64 - r0 : 128 - r0],
                            t_sbs.pop(gd)[:, :, :],
                            start=(r0 == 0),
                            stop=(r0 == 62),
                            perf_mode=DR,
                            tile_position=(0, 0),
                        )
                        if 2 * gd + 1 == 63:
                            softmax_half(0, mid=True)

            # ---- tail: second-half softmax, transpose, aggregate, project ----
            with tc.tile_pool(name="ptail", bufs=2, space="PSUM") as ptail:
                softmax_half(1, mid=False)

                # bf16 transposes relocate row-half h to i-columns 64h..64h+63
                adjnT = const.tile([P, 4, P], BF16)
                for h in range(2):
                    for jt in range(4):
                        pt = ptail.tile([P, 64], BF16, tag="pt")
                        nc.tensor.transpose(
                            pt[:], adjn[:, h, bass.ts(jt, P)], identb_sb
                        )
                        nc.any.tensor_copy(adjnT[:, jt, 64 * h : 64 * h + 64], pt[:])

                aggT_sb = const.tile([P, 2, P], BF16)
                for dh in range(2):
                    pa = ptail.tile([P, P], F32, tag="pa")
                    for jt in range(4):
                        nc.tensor.matmul(
                            pa[:],
                            xb8_sb(jt, dh),
                            adjnT[:, jt, :],
                            start=(jt == 0),
                            stop=(jt == 3),
                        )
                    nc.any.tensor_copy(aggT_sb[:, dh, :], pa[:])

                po = ptail.tile([P, D], F32, tag="po")
                for dh in range(2):
                    nc.tensor.matmul(
                        po[:], aggT_sb[:, dh, :], wg8_sb(dh),
                        start=(dh == 0), stop=(dh == 1),
                    )
                out_sb = const.tile([P, D], F32)
                nc.vector.tensor_tensor(out_sb[:], po[:], bgt_sb, op=OP.add)
                nc.sync.dma_start(out_d[:], out_sb[:])
            loop_cm.__exit__(None, None, None)

    nc.compile()
    return nc


_NC = None


def _get_program():
    global _NC
    if _NC is None:
        _NC = _build_program()
    return _NC


def _core_inputs(x, base_adj, W1, b1, W2, b2, W3, b3, Wg, bg, core):
    b, blk = divmod(core, 4)
    i0 = blk * P
    f32 = np.float32
    bf16 = ml_dtypes.bfloat16
    f8 = ml_dtypes.float8_e4m3

    xbf = np.ascontiguousarray(x[b], dtype=f32)               # [512, 256]
    xtf = np.ascontiguousarray(xbf.T)                         # [256, 512]
    w3 = np.asarray(W3, dtype=f32)[:, 0]                      # [128]

    sgn2 = np.zeros((P, 2, 128), dtype=f32)
    sgn2[:, 0, 64] = np.sign(w3) / TSCALE
    sgn2[:, 1, 65] = np.sign(w3) / TSCALE
    w2s = np.ascontiguousarray(W2.astype(f32) * np.abs(w3)[None, :] * TSCALE)

    bha = 0.5 * base_adj[b, i0 : i0 + P, :].astype(f32)
    eye = np.zeros((P, N), dtype=f32)
    eye[np.arange(P), i0 + np.arange(P)] = 1.0
    bhae = bha + eye
    # pack row-halves onto partitions 0-63 (matching the DoubleRow edge
    # output placement); partitions 64-127 unused
    bhaH = np.zeros((P, 2, N), dtype=f32)
    bhaH[0:64, 0] = bha[0:64]
    bhaH[0:64, 1] = bha[64:128]
    bhaeH = np.zeros((P, 2, N), dtype=f32)
    bhaeH[0:64, 0] = bhae[0:64]
    bhaeH[0:64, 1] = bhae[64:128]

    identb = np.zeros((P, 64), dtype=f32)
    identb[np.arange(64), np.arange(64)] = 1.0

    W1 = np.asarray(W1, f32)
    Wgf = np.asarray(Wg, f32)

    bigh = np.concatenate(
        [
            xtf[:128, :], xtf[128:, :],                        # xt d-tiles
            W1[D:D + 128, :], W1[D + 128 :, :],                # w1b d-tiles
            w2s[:128, :], w2s[128:, :],                        # w2s c-tiles
            xbf[0:128], xbf[128:256], xbf[256:384], xbf[384:], # xb8 j-tiles
            Wgf[:128, :], Wgf[128:, :],                        # wg8 d-tiles
            identb,
        ],
        axis=1,
    ).astype(bf16)
    xtif = np.ascontiguousarray(xbf[i0 : i0 + P, :].T)         # [256, 128]
    bigf = np.concatenate(
        [
            xtif[:128, :], xtif[128:, :],                      # xti d-tiles
            W1[:128, :], W1[128:D, :],                         # w1a d-tiles
            np.asarray(b1, f32).reshape(2, P).T,               # b1c [P, 2]
            (np.abs(w3) * np.asarray(b2, f32) * TSCALE).reshape(P, 1),
            np.full((P, 1), 0.5 * float(np.asarray(b3).reshape(-1)[0]), f32),
        ],
        axis=1,
    )
    bigl = np.concatenate(
        [
            bhaH.reshape(P, 2 * N), bhaeH.reshape(P, 2 * N),
            np.tile(np.asarray(bg, f32)[None, :], (P, 1)),
        ],
        axis=1,
    )
    assert bigh.shape[1] == BH_TOT and bigf.shape[1] == BF_TOT
    assert bigl.shape[1] == BL_TOT
    return {
        "bigh": np.ascontiguousarray(bigh),
        "bigf": np.ascontiguousarray(bigf),
        "bigl": np.ascontiguousarray(bigl),
        "big8": np.ascontiguousarray(sgn2.reshape(P, B8_TOT).astype(f8)),
    }


def run(trace=False, **inputs):
    nc = _get_program()
    inputs = {k: np.asarray(v) for k, v in inputs.items()}
    in_maps = [_core_inputs(core=c, **inputs) for c in range(NCORES)]
    res = run_bass_kernel_spmd(
        nc, in_maps, core_ids=list(range(NCORES)), trace=trace
    )
    out = np.empty((2, N, D), dtype=np.float32)
    for c in range(NCORES):
        b, blk = divmod(c, 4)
        out[b, blk * P : (blk + 1) * P, :] = res.results[c]["out"]
    return out, res


def kernel(**inputs):
    out, _ = run(**inputs)
    return out
